# revision 38
# baseline (speedup 1.0000x reference)
"""AdditiveAttention (Bahdanau) Trainium2 Bass kernel — gapped-ladder v10.

reference:
    Y = tanh(q[:, :, None, :] + k[:, None, :, :])          # [B,Q,K,H]
    scores = einsum("bqkh,h->bqk", Y, w)
    attn = softmax(scores, axis=-1)
    out = einsum("bqk,bkv->bqv", attn, values)             # [B,Q,H]

B=32, Q=256, K=256, H=128.  Data-parallel over batch: 8 cores x 4 batches.
TimelineSim 21373 ns/core (baseline trig-expansion kernel: 33396).
HW rel err 9.0e-3 (gate 2e-2).

Algorithm: tanh(s) ~= sum_n b_n sin(n*a*s), n in {1,3,6,9}, ONE fundamental
a=0.2656 (weighted LSQ over N(0,sqrt2), wrms 1.04e-2).  sin(na(q+k)) =
sin_q cos_k + cos_q sin_k -> an 8-row bf16 matmul contraction.

Why this ladder: only the fundamental touches the ACT Sin table (2 Sin ops
per side; a*|x|+pi/2 <= 3.01 < pi fits the table) — NO range reduction
anywhere.  Harmonic 3 via triple angle from s1^2, 6 via s3*c3 (with the
constant ratio folded into the q-side partner row), 9 via triple angle of 3
— all bf16 DVE tensor_tensor (594ns full / 327 half) and tensor_scalar
(327/194) ops.  Per-row w*coef scales fold into [128,1] ptr slots (wa
columns); additive constants on k-side rows are DROPPED (a per-q score
shift is softmax-invariant), so cos-row affines need only their quadratic
part on the k side.  kc9r folds b9/b3 into IMMs against the scaled kc3r.

Scheduling (iterated against TimelineSim; engine queues are in-order so
emission order = execution order):
  - inputs bf16; DMA order kT, wa, qT, vaug on the serial DMA device
    (each DMA: 625ns HWDGE descriptor + 784 gap + transfer + 900 sem).
  - PE pstate ramp (394/213/107ns per 256-col mm) hidden by ~14 dummy
    512-col warmup matmuls in ONE accumulation group (separate groups get
    WAW semaphores between them, which resets the ramp) spanning the DMA
    wait.
  - ACT: k_s1, q_s1, k_c1, q_c1, kQ3/qQ3 (Square, trig-set), then the one
    Sin->Exp table load (1283ns) lands before the exps while score groups
    are still accumulating; post-compile surgery (_fix_act_loads) rewrites
    the redundant entry set-0 load to the trig set and deletes the
    duplicate so k_s1 starts as soon as kT lands.
  - Pool (slow: ts 1517, tt 2127) takes early k-row scalings ks1r/ks3r,
    ks6r, and the wave-1 halves of the 9-chain affines.
  - DVE runs the ladder k/q-interleaved, then the LATE ops (6/9-chains)
    column-sliced into waves: batches 01 as one half, batches 2 and 3 as
    quarters, so score groups stop staggered and the 8 serial exps (398ns
    each on ACT) pipeline instead of piling up after the last row.
  - phase A score mms row-major (rows 1,3) as rows stream in; phase B
    (rows 6,9) group-major per batch, interleaved with each batch's
    exp -> out-matmul (vaug ones-column gives the softmax denominator) ->
    reciprocal -> DVE out-scale (Pool CANNOT read PSUM on HW; neuronxcc
    rejects it even though the cost model accepts it) -> 3+1 output DMA
    split (b0-b2 combined, b3 alone so the last chain is short).
PSUM: 8 score groups in 8 distinct banks (same-bank group interleave is
broken on HW — verified by the baseline author); warmup + out-stage reuse
freed banks via the same pool ring.
"""

import os

import numpy as np

B, Q, K, H = 32, 256, 256, 128
NCORES = 8
BPC = B // NCORES
PTS = BPC * Q

# {1,3,6,9} * a ladder fit (weighted LSQ, N(0,sqrt2) + 2e-4 floor)
SEED_A = 0.2656
B1, B3, B6, B9 = 1.16182, 0.4142, 0.13754, 0.04259

NWA = 6  # wa ptr columns: wb1, 16wb3, -4wb3, -1024wb6, 256wb9, -12wb9
N_WARM = 14  # PE warmup matmuls (512-col) during input DMA wait

_CACHE: dict = {}


def _build_nc():
    import concourse.bacc as bacc
    import concourse.tile as tile
    from concourse import mybir

    f32 = mybir.dt.float32
    bf16 = mybir.dt.bfloat16
    AF = mybir.ActivationFunctionType
    ALU = mybir.AluOpType
    PI = float(np.pi)

    nc = bacc.Bacc("TRN2", target_bir_lowering=False, debug=False)

    qT_d = nc.dram_tensor("qT", [H, PTS], bf16, kind="ExternalInput")
    kT_d = nc.dram_tensor("kT", [H, PTS], bf16, kind="ExternalInput")
    vaug_d = nc.dram_tensor("vaug", [128, BPC * 2 * 129], bf16, kind="ExternalInput")
    wa_d = nc.dram_tensor("wa", [128, NWA], f32, kind="ExternalInput")
    out_d = nc.dram_tensor("out", [128, BPC * 2 * H], bf16, kind="ExternalOutput")

    with tile.TileContext(nc) as tc:
        with (
            tc.tile_pool(name="const", bufs=1) as cpool,
            tc.tile_pool(name="kf", bufs=1) as kf_pool,
            tc.tile_pool(name="qf", bufs=1) as qf_pool,
            tc.tile_pool(name="expS", bufs=8) as expS_pool,
            tc.tile_pool(name="small", bufs=8) as small_pool,
            tc.tile_pool(name="ps", bufs=8, space="PSUM") as ps_pool,
        ):
            # input DMAs: one serial transfer device — order by need
            kT = cpool.tile([H, PTS], bf16, tag="kT")
            nc.sync.dma_start(kT[:], kT_d.ap()[:, :])
            wa = cpool.tile([128, NWA], f32, tag="wa")
            nc.sync.dma_start(wa[:], wa_d.ap()[:, :])
            qT = cpool.tile([H, PTS], bf16, tag="qT")
            nc.scalar.dma_start(qT[:], qT_d.ap()[:, :])
            vaug = cpool.tile([128, BPC * 2 * 129], bf16, tag="vaug")
            nc.scalar.dma_start(vaug[:], vaug_d.ap()[:, :])

            halfpi = cpool.tile([128, 1], f32, tag="halfpi")
            nc.vector.memset(halfpi[:], PI / 2)
            wjunk = cpool.tile([128, 512], bf16, tag="wjunk")
            nc.gpsimd.memset(wjunk[:], 0.0)

            def wcol(t):
                return wa[:, t : t + 1]

            def ktile(name):
                return kf_pool.tile([H, PTS], bf16, name=name)

            def qtile(name):
                return qf_pool.tile([H, PTS], bf16, name=name)

            def psum_bank():
                return ps_pool.tile([128, 2 * Q], f32, name="psb")

            # ===== PE warmup: ramp the tensor-engine pstate ================
            warm_ps = psum_bank()
            for wi in range(N_WARM):
                nc.tensor.matmul(
                    warm_ps[:, 0:512], wjunk[:, 0:128], wjunk[:, 0:512],
                    start=(wi == 0), stop=(wi == N_WARM - 1),
                )

            # ===== ACT lane: sins then the two squares =====================
            k_s1 = ktile("k_s1")
            nc.scalar.activation(k_s1[:], kT[:], AF.Sin, scale=SEED_A)
            q_s1 = qtile("q_s1")
            nc.scalar.activation(q_s1[:], qT[:], AF.Sin, scale=SEED_A)
            k_c1 = ktile("k_c1")
            nc.scalar.activation(k_c1[:], kT[:], AF.Sin, bias=halfpi[:], scale=SEED_A)
            q_c1 = qtile("q_c1")
            nc.scalar.activation(q_c1[:], qT[:], AF.Sin, bias=halfpi[:], scale=SEED_A)

            # ===== DVE early lane ==========================================
            kP = ktile("kP")
            nc.vector.tensor_tensor(kP[:], k_s1[:], k_s1[:], ALU.mult)
            kt3 = ktile("kt3")
            nc.vector.tensor_scalar(kt3[:], kP[:], 0.75, None, ALU.subtract)
            ks3t = ktile("ks3t")
            nc.vector.tensor_tensor(ks3t[:], kt3[:], k_s1[:], ALU.mult)
            qP = qtile("qP")
            nc.vector.tensor_tensor(qP[:], q_s1[:], q_s1[:], ALU.mult)
            kc1r = ktile("kc1r")  # = wb1 cos1  (ROW)
            nc.vector.tensor_scalar(kc1r[:], k_c1[:], wcol(0), None, ALU.mult)
            qt3 = qtile("qt3")
            nc.vector.tensor_scalar(qt3[:], qP[:], 0.75, None, ALU.subtract)
            qs3t = qtile("qs3t")  # ROW (= -sin3/4)
            nc.vector.tensor_tensor(qs3t[:], qt3[:], q_s1[:], ALU.mult)
            kt3cw = ktile("kt3cw")
            nc.vector.tensor_scalar(kt3cw[:], kP[:], wcol(1), wcol(2), ALU.mult, ALU.add)
            kc3r = ktile("kc3r")  # = -4 wb3 cos3   (ROW)
            nc.vector.tensor_tensor(kc3r[:], kt3cw[:], k_c1[:], ALU.mult)
            qt3c = qtile("qt3c")
            nc.vector.tensor_scalar(qt3c[:], qP[:], 0.25, None, ALU.subtract)
            qc3t = qtile("qc3t")  # ROW (= -cos3/4)
            nc.vector.tensor_tensor(qc3t[:], qt3c[:], q_c1[:], ALU.mult)

            # squares on ACT (Square is in the trig set; the Sin->Exp table
            # load lands after qQ3, before the exps)
            kQ3 = ktile("kQ3")
            nc.scalar.activation(kQ3[:], ks3t[:], AF.Square)
            qQ3 = qtile("qQ3")
            nc.scalar.activation(qQ3[:], qs3t[:], AF.Square)

            # ===== Pool lane ===============================================
            ks1r = ktile("ks1r")  # = wb1 sin1  (ROW)
            nc.gpsimd.tensor_scalar(ks1r[:], k_s1[:], wcol(0), None, ALU.mult)
            ks3r = ktile("ks3r")  # = -4 wb3 sin3   (ROW)
            nc.gpsimd.tensor_scalar(ks3r[:], ks3t[:], wcol(1), None, ALU.mult)
            ks6r = ktile("ks6r")  # = wb3 sin6 / 2 ; q partner carries 2b6/b3
            nc.gpsimd.tensor_tensor(ks6r[:], ks3t[:], kc3r[:], ALU.mult)

            # ===== DVE late lane: halves (batches 01 then 23) ==============
            kc6r = ktile("kc6r")
            qs6t = qtile("qs6t")
            qc6 = qtile("qc6")
            kt9 = ktile("kt9")
            qt9 = qtile("qt9")
            ks9r = ktile("ks9r")
            qs9 = qtile("qs9")
            kt9c2 = ktile("kt9c2")
            kc9r = ktile("kc9r")
            qt9c = qtile("qt9c")
            qc9 = qtile("qc9")

            HC = PTS // 2  # half columns (2 batches)

            # wave-1 halves of the two 9-chain affines go to Pool (it is idle
            # after ks6r and they are needed late)
            s1 = slice(HC, 2 * HC)
            nc.gpsimd.tensor_scalar(
                kt9c2[:, s1], kQ3[:, s1], 16.0 * B9 / B3, B9 / (4.0 * B3),
                ALU.mult, ALU.subtract,
            )
            nc.gpsimd.tensor_scalar(
                qt9c[:, s1], qQ3[:, s1], 256.0, 4.0, ALU.mult, ALU.subtract
            )

            def late_wave(h, s=None):
                if s is None:
                    s = slice(h * HC, (h + 1) * HC)
                nc.vector.tensor_scalar(kc6r[:, s], kQ3[:, s], wcol(3), None, ALU.mult)
                nc.vector.tensor_tensor(qs6t[:, s], qs3t[:, s], qc3t[:, s], ALU.mult)
                nc.vector.tensor_scalar(
                    qc6[:, s], qQ3[:, s], -64.0 * B6 / B3, 2.0 * B6 / B3,
                    ALU.mult, ALU.add,
                )
                nc.vector.tensor_scalar(
                    kt9[:, s], kQ3[:, s], wcol(4), wcol(5), ALU.mult, ALU.add
                )
                nc.vector.tensor_tensor(ks9r[:, s], kt9[:, s], ks3t[:, s], ALU.mult)
                nc.vector.tensor_scalar(
                    qt9[:, s], qQ3[:, s], 256.0, 12.0, ALU.mult, ALU.subtract
                )
                nc.vector.tensor_tensor(qs9[:, s], qt9[:, s], qs3t[:, s], ALU.mult)
                if h == 0:
                    nc.vector.tensor_scalar(
                        kt9c2[:, s], kQ3[:, s], 16.0 * B9 / B3, B9 / (4.0 * B3),
                        ALU.mult, ALU.subtract,
                    )
                nc.vector.tensor_tensor(kc9r[:, s], kt9c2[:, s], kc3r[:, s], ALU.mult)
                if h == 0:
                    nc.vector.tensor_scalar(
                        qt9c[:, s], qQ3[:, s], 256.0, 4.0, ALU.mult, ALU.subtract
                    )
                nc.vector.tensor_tensor(qc9[:, s], qt9c[:, s], qc3t[:, s], ALU.mult)

            QC = PTS // 4
            late_wave(0)
            late_wave(1, slice(2 * QC, 3 * QC))
            late_wave(1, slice(3 * QC, 4 * QC))

            # row pairs (q tile, k tile) in availability order
            PAIRS_A = [(q_s1, kc1r), (qs3t, kc3r), (q_c1, ks1r), (qc3t, ks3r)]
            PAIRS_B = [(qs6t, kc6r), (qc6, ks6r), (qs9, kc9r), (qc9, ks9r)]

            # ===== scores ==================================================
            groups = [(b, chunk) for b in range(BPC) for chunk in range(2)]
            scores_ps = {g: psum_bank() for g in groups}

            def score_mm(b, chunk, qt, kt, start, stop):
                nc.tensor.matmul(
                    scores_ps[(b, chunk)][:, 0:Q],
                    kt[:, b * K + chunk * 128 : b * K + chunk * 128 + 128],
                    qt[:, b * Q : (b + 1) * Q],
                    start=start,
                    stop=stop,
                )

            for mi, (qt, kt) in enumerate(PAIRS_A):
                for b, chunk in groups:
                    score_mm(b, chunk, qt, kt, mi == 0, False)

            # phase B group-major per batch, interleaved with the out stage:
            # B-b0, B-b1, out-b0, B-b2, out-b1, B-b3, out-b2, out-b3
            eS = {}
            osb_all = cpool.tile([128, BPC * 2 * H], bf16, tag="osb_all")

            def emit_B(b):
                for chunk in range(2):
                    for mi, (qt, kt) in enumerate(PAIRS_B):
                        score_mm(b, chunk, qt, kt, False, mi == len(PAIRS_B) - 1)
                for chunk in range(2):
                    e = expS_pool.tile([128, Q], bf16, name="eS")
                    nc.scalar.activation(e[:], scores_ps[(b, chunk)][:, 0:Q], AF.Exp)
                    eS[(b, chunk)] = e

            def emit_out(b):
                for qb in range(2):
                    outp = psum_bank()
                    for chunk in range(2):
                        nc.tensor.matmul(
                            outp[:, 0:129],
                            eS[(b, chunk)][:, qb * 128 : qb * 128 + 128],
                            vaug[:, (b * 2 + chunk) * 129 : (b * 2 + chunk + 1) * 129],
                            start=(chunk == 0),
                            stop=(chunk == 1),
                        )
                    recip = small_pool.tile([128, 1], f32)
                    nc.vector.reciprocal(recip[:], outp[:, 128:129])
                    g = b * 2 + qb
                    dst = osb_all[:, g * H : (g + 1) * H]
                    nc.vector.tensor_scalar(dst, outp[:, 0:128], recip[:], None, ALU.mult)

            def out_dma(b):
                nc.sync.dma_start(
                    out_d.ap()[:, b * 2 * H : (b + 1) * 2 * H],
                    osb_all[:, b * 2 * H : (b + 1) * 2 * H],
                )

            def out_dma(b):
                nc.sync.dma_start(
                    out_d.ap()[:, b * 2 * H : (b + 1) * 2 * H],
                    osb_all[:, b * 2 * H : (b + 1) * 2 * H],
                )

            emit_B(0)
            emit_B(1)
            emit_out(0)
            emit_B(2)
            emit_out(1)
            emit_B(3)
            emit_out(2)
            nc.sync.dma_start(out_d.ap()[:, 0 : 6 * H], osb_all[:, 0 : 6 * H])
            emit_out(3)
            out_dma(3)

    nc.compile()
    _fix_act_loads(nc)
    return nc


def _fix_act_loads(nc):
    """The act-table pass emits [set0, set9] back-to-back at entry plus the
    mid-kernel set0 (exp) load.  The entry set0 load is dead weight on the
    ACT queue right when the sins want to start: rewrite it to set9 and drop
    the now-duplicate second load."""
    import concourse.mybir as mybir

    fn = nc.m.functions[0]
    for blk in fn.blocks:
        loads = [
            i for i in blk.instructions if isinstance(i, mybir.InstLoadActFuncSet)
        ]
        if len(loads) >= 2 and [l.act_func_set_id for l in loads[:2]] == [0, 9]:
            loads[0].act_func_set_id = 9
            blk.instructions = [i for i in blk.instructions if i is not loads[1]]


def _get_nc():
    if "nc" not in _CACHE:
        _CACHE["nc"] = _build_nc()
    return _CACHE["nc"]


def _prep_core_inputs(queries, keys, values, w, c):
    import ml_dtypes

    bs = slice(c * BPC, (c + 1) * BPC)
    qT = np.ascontiguousarray(
        queries[bs].transpose(2, 0, 1).reshape(H, PTS)
    ).astype(ml_dtypes.bfloat16)
    kT = np.ascontiguousarray(
        keys[bs].transpose(2, 0, 1).reshape(H, PTS)
    ).astype(ml_dtypes.bfloat16)
    va = np.ones((BPC, 2, 128, 129), dtype=np.float32)
    va[..., :128] = values[bs].reshape(BPC, 2, 128, 128)
    vaug = np.ascontiguousarray(
        va.transpose(2, 0, 1, 3).reshape(128, BPC * 2 * 129)
    ).astype(ml_dtypes.bfloat16)
    wa = np.empty((128, NWA), dtype=np.float32)
    wa[:, 0] = w * B1
    wa[:, 1] = 16.0 * w * B3
    wa[:, 2] = -4.0 * w * B3
    wa[:, 3] = -1024.0 * w * B6
    wa[:, 4] = 256.0 * w * B9
    wa[:, 5] = -12.0 * w * B9
    return {"qT": qT, "kT": kT, "vaug": vaug, "wa": wa}


def kernel(queries, keys, values, w):
    from concourse.bass_utils import run_bass_kernel_spmd
    from concourse._compat import axon_active

    if os.environ.get("BASS_TRACE") and axon_active():
        try:
            import antenv.axon_hooks  # noqa: F401
        except ImportError:
            os.environ["BASS_NEVER_TRACE"] = "1"

    queries = np.asarray(queries, dtype=np.float32)
    keys = np.asarray(keys, dtype=np.float32)
    values = np.asarray(values, dtype=np.float32)
    w = np.asarray(w, dtype=np.float32)

    nc = _get_nc()
    in_maps = [_prep_core_inputs(queries, keys, values, w, c) for c in range(NCORES)]
    res = run_bass_kernel_spmd(nc, in_maps, core_ids=list(range(NCORES)))
    _CACHE["last_result"] = res
    outs = []
    for c in range(NCORES):
        o = np.asarray(res.results[c]["out"], dtype=np.float32)  # [128, BPC*2*H]
        o = o.reshape(128, BPC * 2, H).transpose(1, 0, 2).reshape(BPC * Q, H)
        outs.append(o)
    out = np.concatenate(outs, axis=0)
    return out.reshape(B, Q, H)
